# revision 18
# baseline (speedup 1.0000x reference)
"""MoD-router FFN kernel for 8 TRN2 NeuronCores (self-contained).

Math note: the reference applies softmax over a size-1 axis, which yields
all-ones scores for ANY input; jax.lax.top_k is stable, so the selected
token indices are always [0..NUM_TOKENS) per batch row. The router weights
(Wp, bp) therefore cannot affect the output, and the kernel computes

    out = gelu_tanh(x[:, :2048, :] @ W1 + b1) @ W2 + b2

Sharding: data-parallel over the 4*2048 = 8192 selected token rows ->
1024 rows per core. Each core runs a fused transposed FFN:
  H^T = gelu(W1^T @ X^T + b1)   (per F-block of 512, kept in SBUF)
  out^T += W2_blk^T @ H^T_blk   (accumulated in SBUF fp32)
Matmuls run in bfloat16 (same PE rate as fp32r, but FWL halves the
LDWEIGHTS stream so weight loads hide fully behind the matmuls, and all
DMA/SBUF traffic halves; measured ~3e-3 rel err).

DMA queueing: each dma_start occupies its issuing engine queue ~0.6us, so
input streaming is split across two HWDGE rings - weights on the sync
queue, activations (xt) batched up to 1 MB on the scalar queue. Output
stores issue per 512-row chunk right after its accumulate, alternating
rings, so the 8 MB writeback flows during the whole final block instead
of serializing behind the last w2 loads (in-order queues).
"""

import numpy as np

B, S, D, F = 4, 4096, 2048, 8192
NUM_TOKENS = 2048
NCORES = 8
ROWS = (B * NUM_TOKENS) // NCORES     # 1024 rows per core
P = 128
KT_D = D // P                         # 16 k-subtiles over D
FT = F // P                           # 64 f-tiles
FB = 16                               # F-blocks of 512
FSUB = 4                              # f-subtiles per block
DT = D // P                           # 16 d-tiles
NCH = ROWS // 512                     # 2 row chunks of 512
KS_W2 = 4                             # k-subtiles per F-block in FFN2

_CACHE = {}


def _build():
    import concourse.mybir as mybir
    import concourse.tile as tile
    from concourse import bacc

    f32 = mybir.dt.float32
    bf16 = mybir.dt.bfloat16

    nc = bacc.Bacc()
    xt = nc.declare_dram_parameter("xt", [P, KT_D * ROWS], bf16, isOutput=False)
    w1 = nc.declare_dram_parameter("w1", [FT, P, KT_D, P], bf16, isOutput=False)
    w2 = nc.declare_dram_parameter("w2", [FB, DT, P, KS_W2, P], bf16, isOutput=False)
    b1 = nc.declare_dram_parameter("b1", [P, FT], f32, isOutput=False)
    b2 = nc.declare_dram_parameter("b2", [P, DT], f32, isOutput=False)
    out = nc.declare_dram_parameter("out", [DT, P, ROWS], f32, isOutput=True)

    with tile.TileContext(nc) as tc:
        with (
            tc.tile_pool(name="resident", bufs=1) as res_pool,
            tc.tile_pool(name="w1p", bufs=6) as w1p,
            tc.tile_pool(name="w2p", bufs=6) as w2p,
            tc.tile_pool(name="htp", bufs=8) as htp,
            tc.tile_pool(name="ps1", bufs=4, space="PSUM") as ps1,
            tc.tile_pool(name="ps2", bufs=4, space="PSUM") as ps2,
        ):
            # xt lives in one contiguous SBUF tile so k-batches load with a
            # single large DMA; slice k via xs().
            xt_sb = res_pool.tile([P, KT_D * ROWS], bf16, name="xtsb")

            def xs(k, lo, hi):
                return xt_sb[:, k * ROWS + lo:k * ROWS + hi]

            w1_warm = [w1p.tile([P, KT_D * P], bf16, name=f"w1t_{ft}", tag="w1t")
                       for ft in range(FSUB)]
            b1_sb = res_pool.tile([P, FT], f32, name="b1sb")
            b2_sb = res_pool.tile([P, DT], f32, name="b2sb")

            # zeroed operands for the HAM pre-warm matmuls below
            dum_w = res_pool.tile([P, P], bf16, name="dumw")
            dum_x = res_pool.tile([P, 512], bf16, name="dumx")
            nc.gpsimd.memset(dum_w[:], 0.0)
            nc.gpsimd.memset(dum_x[:], 0.0)

            # xt streams on the scalar HWDGE ring, batched; the warmup block
            # consumes k in order so the early batches are small for latency
            # and the later ones big for issue-rate. The k>=8 tail batches,
            # biases, and later weights go at the back of the sync ring so
            # the HBM round-robin doesn't starve the first-needed pieces.
            nc.scalar.dma_start(out=xs(0, 0, 512), in_=xt[:, 0:512])
            nc.scalar.dma_start(out=xs(0, 512, ROWS), in_=xt[:, 512:ROWS])
            nc.scalar.dma_start(out=xs(1, 0, ROWS), in_=xt[:, ROWS:2 * ROWS])
            for k0, k1 in ((2, 4), (4, 8), (8, 12)):
                nc.scalar.dma_start(
                    out=xt_sb[:, k0 * ROWS:k1 * ROWS],
                    in_=xt[:, k0 * ROWS:k1 * ROWS])

            # w1 warmup tiles stream on the sync ring in k-consumption order.
            def w1chunk(ft, i):  # k-slices 4i..4i+3 of warm tile ft
                nc.sync.dma_start(
                    out=w1_warm[ft][:, i * 4 * P:(i + 1) * 4 * P],
                    in_=w1[ft, :, i * 4:(i + 1) * 4, :].rearrange("p k c -> p (k c)"))

            for i in range(4):
                for ft in range(FSUB):
                    w1chunk(ft, i)
            nc.sync.dma_start(out=b1_sb[:], in_=b1[:])
            nc.sync.dma_start(out=b2_sb[:], in_=b2[:])
            nc.sync.dma_start(out=xt_sb[:, 12 * ROWS:16 * ROWS],
                              in_=xt[:, 12 * ROWS:16 * ROWS])

            # out accumulator; fb==0's FFN2 initializes it via ScalarE with
            # the b2 bias folded in, later blocks accumulate on VectorE.
            oacc = [res_pool.tile([P, ROWS], f32, name=f"oacc{d}") for d in range(DT)]

            for fb in range(FB):
                ht = []
                if fb == 0:
                    # warmup block: k-outer over all 8 (fs, n) chains (uses
                    # all 8 PSUM banks) so matmuls start as soon as the xt
                    # k-slices land instead of waiting for all of XT.
                    for fs in range(FSUB):
                        ht.append(htp.tile([P, ROWS], bf16, name=f"ht_{fs}", tag="ht"))
                    # n-outer so the first 4 matmuls of each k need only the
                    # first 512-row xt half (the n=1 half lands meanwhile)
                    chains = [(fs, n) for n in range(NCH) for fs in range(FSUB)]
                    psums = {}
                    for fs, n in chains:
                        pool, tag = (ps1, "ps1") if fs < 2 else (ps2, "ps2")
                        psums[(fs, n)] = pool.tile([P, 512], f32,
                                                   name=f"psw_{fs}_{n}", tag=tag)
                    # HAM pre-warm: dependency-free matmuls run while the
                    # first DMAs are still in flight, keeping the PE activity
                    # window continuously busy so the 1.2->2.4 GHz unthrottle
                    # fires ~6us earlier. They overwrite psums[(0,0)] before
                    # its real chain starts (WAW, cleared by its start=True).
                    for _ in range(8):
                        nc.tensor.matmul(psums[(0, 0)][:], dum_w[:], dum_x[:],
                                         start=True, stop=True)
                    for k in range(KT_D):
                        for fs, n in chains:
                            nc.tensor.matmul(
                                psums[(fs, n)][:],
                                w1_warm[fs][:, k * P:(k + 1) * P],
                                xs(k, n * 512, (n + 1) * 512),
                                start=(k == 0), stop=(k == KT_D - 1),
                            )
                    for fs, n in chains:
                        nc.scalar.activation(
                            ht[fs][:, n * 512:(n + 1) * 512], psums[(fs, n)][:],
                            mybir.ActivationFunctionType.Gelu_apprx_tanh,
                            bias=b1_sb[:, fs:fs + 1],
                        )
                else:
                    for fs in range(FSUB):
                        ft = fb * FSUB + fs
                        w1_sb = w1p.tile([P, KT_D * P], bf16, name=f"w1t_{ft}", tag="w1t")
                        nc.sync.dma_start(out=w1_sb[:], in_=w1[ft].rearrange("p k c -> p (k c)"))
                        ht_t = htp.tile([P, ROWS], bf16, name=f"ht_{ft}", tag="ht")
                        for n in range(NCH):
                            psum = ps1.tile([P, 512], f32, name=f"ps1_{ft}_{n}", tag="ps1")
                            for k in range(KT_D):
                                nc.tensor.matmul(
                                    psum[:],
                                    w1_sb[:, k * P:(k + 1) * P],
                                    xs(k, n * 512, (n + 1) * 512),
                                    start=(k == 0), stop=(k == KT_D - 1),
                                )
                            nc.scalar.activation(
                                ht_t[:, n * 512:(n + 1) * 512], psum[:],
                                mybir.ActivationFunctionType.Gelu_apprx_tanh,
                                bias=b1_sb[:, ft:ft + 1],
                            )
                        ht.append(ht_t)

                for d in range(DT):
                    w2_sb = w2p.tile([P, KS_W2 * P], bf16, name=f"w2t_{fb}_{d}", tag="w2t")
                    nc.sync.dma_start(out=w2_sb[:], in_=w2[fb, d].rearrange("p k c -> p (k c)"))
                    for n in range(NCH):
                        psum2 = ps2.tile([P, 512], f32, name=f"ps2_{fb}_{d}_{n}", tag="ps2")
                        for ks in range(KS_W2):
                            nc.tensor.matmul(
                                psum2[:],
                                w2_sb[:, ks * P:(ks + 1) * P],
                                ht[ks][:, n * 512:(n + 1) * 512],
                                start=(ks == 0), stop=(ks == KS_W2 - 1),
                            )
                        if fb == 0:
                            nc.scalar.activation(
                                oacc[d][:, n * 512:(n + 1) * 512], psum2[:],
                                mybir.ActivationFunctionType.Identity,
                                bias=b2_sb[:, d:d + 1],
                            )
                        elif fb == FB - 1 and d == DT - 1 and n == NCH - 1:
                            # the kernel's tail: halve the last accumulate +
                            # store so the final writeback chunk is smaller
                            # and the two halves drain on both rings.
                            for h, eng in ((0, nc.scalar), (1, nc.sync)):
                                lo, hi = n * 512 + h * 256, n * 512 + (h + 1) * 256
                                nc.vector.tensor_add(
                                    oacc[d][:, lo:hi], oacc[d][:, lo:hi],
                                    psum2[:, h * 256:(h + 1) * 256],
                                )
                                eng.dma_start(out=out[d, :, lo:hi],
                                              in_=oacc[d][:, lo:hi])
                        else:
                            nc.vector.tensor_add(
                                oacc[d][:, n * 512:(n + 1) * 512],
                                oacc[d][:, n * 512:(n + 1) * 512],
                                psum2[:],
                            )
                        if fb == FB - 1 and not (d == DT - 1 and n == NCH - 1):
                            # store each chunk as soon as its accumulate
                            # lands, alternating rings so the 8 MB writeback
                            # keeps pace with the final block's compute.
                            eng = nc.scalar if d % 2 == 0 else nc.sync
                            eng.dma_start(out=out[d, :, n * 512:(n + 1) * 512],
                                          in_=oacc[d][:, n * 512:(n + 1) * 512])

    nc.compile()
    return nc


def _get_nc():
    if "nc" not in _CACHE:
        _CACHE["nc"] = _build()
    return _CACHE["nc"]


def _prep_in_maps(x, W1, b1, W2, b2):
    """Host-side shard + layout prep (bf16 weights/activations)."""
    import ml_dtypes

    bf = ml_dtypes.bfloat16
    xs = x[:, :NUM_TOKENS, :].reshape(B * NUM_TOKENS, D)         # [8192, 2048]
    w1h = W1.reshape(KT_D, P, FT, P).transpose(2, 1, 0, 3).astype(bf)
    w2h = W2.reshape(FB, KS_W2, P, DT, P).transpose(0, 3, 2, 1, 4).astype(bf)
    b1h = np.ascontiguousarray(b1.reshape(FT, P).T)              # [p, ft]
    b2h = np.ascontiguousarray(b2.reshape(DT, P).T)              # [p, d]

    in_maps = []
    for c in range(NCORES):
        xc = xs[c * ROWS:(c + 1) * ROWS]                         # [1024, 2048]
        # partition-major [p, (k n)] so k-batches are contiguous DMA slices
        xth = (xc.T.reshape(KT_D, P, ROWS).transpose(1, 0, 2)
               .reshape(P, KT_D * ROWS).astype(bf))
        in_maps.append({"xt": xth, "w1": w1h, "w2": w2h, "b1": b1h, "b2": b2h})
    return in_maps


def _gather(results):
    out = np.empty((B * NUM_TOKENS, D), dtype=np.float32)
    for c in range(NCORES):
        oc = results[c]["out"]                                   # [d, p, n]
        out[c * ROWS:(c + 1) * ROWS] = oc.reshape(D, ROWS).T
    return out.reshape(B, NUM_TOKENS, D)


def kernel(x, Wp, bp, W1, b1, W2, b2, **_unused):
    from concourse.bass_utils import run_bass_kernel_spmd

    x = np.asarray(x, dtype=np.float32)
    W1 = np.asarray(W1, dtype=np.float32)
    W2 = np.asarray(W2, dtype=np.float32)
    b1 = np.asarray(b1, dtype=np.float32)
    b2 = np.asarray(b2, dtype=np.float32)

    in_maps = _prep_in_maps(x, W1, b1, W2, b2)
    nc = _get_nc()
    res = run_bass_kernel_spmd(nc, in_maps, list(range(NCORES)))
    return _gather(res.results)
